# revision 1
# baseline (speedup 1.0000x reference)
"""Trainium2 Bass kernel for nn_CCELoss (calibration-histogram loss).

Sharding: data-parallel over image rows, 8 NeuronCores, 128 rows each.

Per-core layout: logits as [114 = 6 pixel-groups x 19 classes, F=45056]
(group g covers core-flat pixels [g*F, (g+1)*F); tail of group 5 is padding
with logit 0 -> p = 1/19 -> bin 0, corrected on host).

Per 4096-pixel tile:
  ACT  e = exp(l)
  PE   Z[g,n] = sum_c e[(g,c),n]          (block-diag ones matmul, fp32)
  DMA  reshape Z [6,2048] -> [96,128] pixel-major
  ACT  m = ln(Z)                          (Exp/Ln share one ACT table set)
  DMA  reshape back -> [6,2048]
  PE   mb = broadcast m to [114, .]       (block-diag ones matmul)
  DVE  d = l - mb   (in-place over l)
  ACT  p = exp(d)
  folds with fused accumulate, split across DVE and ACT:
    counts N_i = sum [p > i/10]           DVE tensor_scalar(is_gt)
    conf   R_i = sum relu(p - i/10)       DVE max/sub or ACT Relu(bias=-t)
True-class side channel (for the accuracy histogram): host passes the
gathered true-class logit l* in the same pixel-major [96, .] layout;
d* = l* - m, p* = exp(d*) is bit-identical to p at the true class and is
returned to the host, which bins it against target (tiny).
Host: decode folds -> conf/count hists, bin p* -> acc hist, loss formula.
"""

import numpy as np

import bass_rust
import concourse.bass as bass
from concourse import bacc
import concourse.mybir as mybir
import concourse.tile as tile
from concourse.vector_clock import ScopedClock
from concourse.bass_utils import run_bass_kernel_spmd

F32 = mybir.dt.float32
AF = mybir.ActivationFunctionType
ALU = mybir.AluOpType

# ---------------- problem geometry (hardcoded) ----------------
C = 19
NB = 10
H, W = 1024, 2048
NCORES = 8
ROWS = H // NCORES          # 128
NPIX = ROWS * W             # 262144 valid pixels per core
G = 6
P = G * C                   # 114 partitions
TILE_F = 4096
NT = 11
F = NT * TILE_F             # 45056
NPAD = G * F - NPIX         # 8192 pad pixels
VALID_J5 = NPIX - 5 * F     # 36864 valid pixels in group 5
PAD_TILE0 = VALID_J5 // TILE_F  # = 9; tiles 9,10 have group 5 all-pad

THR = [np.float32(i / 10.0) for i in range(10)]
N_CONF = 10
N_CNT = 9
NFOLD = N_CONF + N_CNT      # 19 fold slots per tile
DVE_FOLD_CONF = 3           # conf folds 0..2 on DVE, 3..9 on ACT

MM_CHUNK = 512              # fp32 moving-operand limit
PCOLS = NT * 256            # pixel-major cols: (t, h, c128) -> t*256+h*128+c

_BUILD_CACHE = {}


def _patch_tile_drain():
    """walrus rejects drains with >1 sync wait; split the tile-exit drain."""
    if getattr(tile.TileContext, "_drain_patched", False):
        return

    def _drain_and_barrier(self, tick_clock, wait_clock):
        drain_inst = self.nc.sync.drain()
        wait_clock.add_sem_waits(
            drain_inst.ins, ScopedClock({None: tick_clock.global_clock})
        )
        si = drain_inst.ins.sync_info
        if si is not None and len(si.on_wait) > 1:
            waits = list(si.on_wait)
            ups = list(si.on_update)
            drain_inst.ins.sync_info = mybir.SyncInfo(on_wait=waits[:1], on_update=[])
            last = drain_inst
            for i in range(1, len(waits)):
                last = self.nc.sync.drain()
                last.ins.sync_info = mybir.SyncInfo(on_wait=waits[i:i + 1], on_update=[])
            if ups:
                lw = list(last.ins.sync_info.on_wait) if last.ins.sync_info else []
                last.ins.sync_info = mybir.SyncInfo(on_wait=lw, on_update=ups)
        self.nc.all_engine_barrier()
        assert self.sems is not None
        popped = self.nc._tile_sem_poison_stack.pop()
        assert popped is self._sem_poison
        self.nc.clear_and_free_semaphores(list(self.sems.allocated().values()))
        self.nc.all_engine_barrier()

    tile.TileContext._drain_and_barrier = _drain_and_barrier
    tile.TileContext._drain_patched = True


def build_nc():
    _patch_tile_drain()
    nc = bacc.Bacc()

    # register threshold constants (+/-t_i) as const APs
    for i in range(1, 10):
        for v in (float(-THR[i]), float(THR[i])):
            if (F32, v) in nc.const_aps.aps:
                continue
            tns = nc.alloc_sbuf_tensor(f"const-thr-{v}", [128, 1], F32)
            nc.gpsimd.memset(tns.ap(), v)
            nc.const_aps.aps[(F32, v)] = tns.ap()
    nc.all_engine_barrier()

    lg = nc.declare_dram_parameter("lg", [C, NPIX], F32, isOutput=False)
    zpad = nc.declare_dram_parameter("zpad", [C, TILE_F], F32, isOutput=False)
    lstar = nc.declare_dram_parameter("lstar", [96, PCOLS], F32, isOutput=False)
    bdones = nc.declare_dram_parameter("bdones", [P, G], F32, isOutput=False)
    bcast = nc.declare_dram_parameter("bcast", [G, P], F32, isOutput=False)
    folds_out = nc.declare_dram_parameter("folds", [P, NT * NFOLD], F32, isOutput=True)
    pstar_out = nc.declare_dram_parameter("pstar", [96, PCOLS], F32, isOutput=True)

    with tile.TileContext(nc) as tc:
        with (
            tc.tile_pool(name="const", bufs=1) as constp,
            tc.tile_pool(name="lt", bufs=2) as lp,
            tc.tile_pool(name="et", bufs=2) as ep,
            tc.tile_pool(name="pt", bufs=2) as pp,
            tc.tile_pool(name="mc", bufs=2) as mcp,
            tc.tile_pool(name="mt", bufs=2) as mp,
            tc.tile_pool(name="lst", bufs=2) as lsp,
            tc.tile_pool(name="acc", bufs=1) as accp,
            tc.tile_pool(name="zpsum", bufs=1, space="PSUM") as zp,
            tc.tile_pool(name="mbpsum", bufs=2, space="PSUM") as mbp,
        ):
            bd_sb = constp.tile([P, G], F32)
            nc.gpsimd.dma_start(out=bd_sb[:], in_=bdones[:])
            bc_sb = constp.tile([G, P], F32)
            nc.gpsimd.dma_start(out=bc_sb[:], in_=bcast[:])

            foldacc = accp.tile([P, NT * NFOLD], F32)
            dstar = accp.tile([96, PCOLS], F32)
            scr_dve = accp.tile([P, TILE_F], F32)
            scr_act = accp.tile([P, TILE_F], F32)

            for t in range(NT):
                # ---- load logits tile [114, TILE_F] ----
                lt = lp.tile([P, TILE_F], F32)
                ng = G if t < PAD_TILE0 else G - 1
                base = lg[:, t * TILE_F:(t + 1) * TILE_F]
                src3 = bass_rust.AP(
                    tensor=base.tensor, offset=base.offset,
                    ap=[[F, ng]] + list(base.ap))
                nc.gpsimd.dma_start(out=lt[0:C * ng, :], in_=src3)
                if ng < G:
                    nc.gpsimd.dma_start(out=lt[C * 5:P, :], in_=zpad[:])

                # ---- e = exp(l) ----
                et = ep.tile([P, TILE_F], F32)
                nc.scalar.activation(et[:], lt[:], AF.Exp)

                # ---- l* tile (pixel-major) ----
                lst = lsp.tile([96, 256], F32)
                nc.gpsimd.dma_start(out=lst[:], in_=lstar[:, t * 256:(t + 1) * 256])

                # ---- per 2048-half: Z, ln, broadcast, d = l - mb ----
                for h in range(2):
                    zps = zp.tile([G, 2048], F32)
                    for q in range(4):
                        c0 = h * 2048 + q * MM_CHUNK
                        nc.tensor.matmul(
                            zps[:, q * MM_CHUNK:(q + 1) * MM_CHUNK],
                            bd_sb[:],
                            et[:, c0:c0 + MM_CHUNK],
                            start=True, stop=True,
                        )
                    # m = ln(Z): ACT reads PSUM directly, writes [6, 2048]
                    mt = mp.tile([G, 2048], F32)
                    nc.scalar.activation(mt[:], zps[:], AF.Ln)
                    # pixel-major copy of m for the true-class side channel
                    mc = mcp.tile([96, 128], F32)
                    nc.gpsimd.dma_start(
                        out=mc[:],
                        in_=mt[:].rearrange("g (r c) -> g r c", r=16),
                    )
                    # d* = l* - m  (pixel-major [96, 128])
                    nc.vector.tensor_sub(
                        dstar[:, t * 256 + h * 128:t * 256 + (h + 1) * 128],
                        lst[:, h * 128:(h + 1) * 128],
                        mc[:],
                    )
                    for hh in range(2):
                        mb = mbp.tile([P, 1024], F32)
                        for q in range(2):
                            c0 = hh * 1024 + q * MM_CHUNK
                            nc.tensor.matmul(
                                mb[:, q * MM_CHUNK:(q + 1) * MM_CHUNK],
                                bc_sb[:],
                                mt[:, c0:c0 + MM_CHUNK],
                                start=True, stop=True,
                            )
                        d0 = h * 2048 + hh * 1024
                        nc.vector.tensor_sub(
                            lt[:, d0:d0 + 1024], lt[:, d0:d0 + 1024], mb[:],
                        )

                # ---- p = exp(d) ----
                pt = pp.tile([P, TILE_F], F32)
                nc.scalar.activation(pt[:], lt[:], AF.Exp)

                # ---- folds ----
                base = t * NFOLD
                for i in range(1, 10):   # counts on DVE: accum = sum [p > t]
                    nc.vector.tensor_scalar(
                        scr_dve[:], pt[:], float(THR[i]), None, ALU.is_gt,
                        ALU.add,
                        accum_out=foldacc[:, base + N_CONF + i - 1:base + N_CONF + i],
                    )
                for i in range(10):      # conf folds
                    col = foldacc[:, base + i:base + i + 1]
                    if i == 0:
                        # accum = sum max(p, 0) = sum p
                        nc.vector.tensor_scalar(
                            scr_dve[:], pt[:], 0.0, None,
                            ALU.max, ALU.add, accum_out=col,
                        )
                    elif i < DVE_FOLD_CONF:
                        # accum = sum (max(p, t) - t) = sum relu(p - t)
                        nc.vector.scalar_tensor_tensor(
                            scr_dve[:], pt[:], float(THR[i]),
                            nc.const_aps.tensor(float(THR[i]), [P, TILE_F]),
                            ALU.max, ALU.subtract, accum_out=col,
                        )
                    else:
                        nc.scalar.activation(
                            scr_act[:], pt[:], AF.Relu,
                            bias=-float(THR[i]), accum_out=col,
                        )

            # ---- end phase ----
            pstar_sb = accp.tile([96, PCOLS], F32)
            nc.scalar.activation(pstar_sb[:], dstar[:], AF.Exp)
            nc.gpsimd.dma_start(out=pstar_out[:], in_=pstar_sb[:])
            nc.gpsimd.dma_start(out=folds_out[:], in_=foldacc[:])

    nc.finalize()
    return nc


def _make_consts():
    bd = np.zeros((P, G), np.float32)
    bc = np.zeros((G, P), np.float32)
    for g in range(G):
        bd[C * g:C * (g + 1), g] = 1.0
        bc[g, C * g:C * (g + 1)] = 1.0
    return bd, bc


def _shard_host(output: np.ndarray, target: np.ndarray):
    o = np.ascontiguousarray(output[0])          # [19, 1024, 2048]
    t = np.ascontiguousarray(target[0])          # [1024, 2048]
    lstar_full = np.take_along_axis(o, t[None], axis=0)[0]
    bd, bc = _make_consts()

    in_maps = []
    for core in range(NCORES):
        r0 = core * ROWS
        lg = np.ascontiguousarray(o[:, r0:r0 + ROWS, :].reshape(C, NPIX))
        ls = lstar_full[r0:r0 + ROWS, :].reshape(-1)
        ls = np.concatenate([ls, np.zeros(NPAD, np.float32)])
        # flat n = g*F + t*4096 + h*2048 + r*128 + c  ->  [96=(g,r), t*256+h*128+c]
        ls = (ls.reshape(G, NT, 2, 16, 128).transpose(0, 3, 1, 2, 4)
                .reshape(96, PCOLS))
        in_maps.append({
            "lg": lg, "lstar": np.ascontiguousarray(ls),
            "bdones": bd, "bcast": bc,
            "zpad": np.zeros((C, TILE_F), np.float32),
        })
    return in_maps


def _pstar_to_flat(ps: np.ndarray) -> np.ndarray:
    """[96, PCOLS] pixel-major -> core-flat [G*F] (incl pad)."""
    return (ps.reshape(G, 16, NT, 2, 128).transpose(0, 2, 3, 1, 4).reshape(-1))


def _decode_and_loss(results, target: np.ndarray):
    conf = np.zeros((C, NB), np.float64)
    cnt = np.zeros((C, NB), np.float64)
    acc = np.zeros((C, NB), np.float64)

    # device-replicated pad probability: p_pad = exp(0 - ln(19*exp(0)))
    p_pad = np.float32(np.exp(np.float32(-np.log(np.float32(19.0)))))

    for core in range(NCORES):
        folds = results[core]["folds"].astype(np.float64)
        folds = folds.reshape(P, NT, NFOLD).sum(axis=1)      # [114, 19]
        R = folds[:, :N_CONF].reshape(G, C, N_CONF).sum(axis=0)   # [C, 10]
        Ni = folds[:, N_CONF:].reshape(G, C, N_CNT).sum(axis=0)   # [C, 9]

        R[:, 0] -= NPAD * np.float64(p_pad)      # pad contributes only to R_0
        Ni = np.concatenate([np.full((C, 1), float(NPIX)), Ni], axis=1)

        tgrid = np.arange(10, dtype=np.float64) / 10.0
        S = R + tgrid[None, :] * Ni              # S_i = sum p * [p > t_i]
        Snext = np.concatenate([S[:, 1:], np.zeros((C, 1))], axis=1)
        Nnext = np.concatenate([Ni[:, 1:], np.zeros((C, 1))], axis=1)
        conf += S - Snext
        cnt += Ni - Nnext

        r0 = core * ROWS
        ps = _pstar_to_flat(results[core]["pstar"])[:NPIX]
        y = target[0, r0:r0 + ROWS, :].reshape(-1)
        b = np.clip(np.ceil(ps * np.float32(10.0)).astype(np.int32) - 1, 0, NB - 1)
        acc += np.bincount(y * NB + b, minlength=C * NB).reshape(C, NB)

    EPS = 1e-13
    avg_acc = acc / (cnt + EPS)
    avg_conf = conf / (cnt + EPS)
    loss = np.sum((avg_acc - avg_conf) ** 2 * (cnt / cnt.sum()))
    return np.float32(loss), (conf, cnt, acc)


def kernel(output: np.ndarray, target: np.ndarray) -> np.ndarray:
    output = np.asarray(output, np.float32)
    target = np.asarray(target, np.int32)
    if "nc" not in _BUILD_CACHE:
        _BUILD_CACHE["nc"] = build_nc()
    nc = _BUILD_CACHE["nc"]
    in_maps = _shard_host(output, target)
    res = run_bass_kernel_spmd(nc, in_maps, list(range(NCORES)))
    loss, _ = _decode_and_loss(res.results, target)
    return np.float32(loss)



# revision 4
# speedup vs baseline: 1.4617x; 1.4617x over previous
"""Trainium2 Bass kernel for nn_CCELoss (calibration-histogram loss).

Sharding: data-parallel over image rows, 8 NeuronCores, 128 rows each.

Per-core layout: logits as [114 = 6 pixel-groups x 19 classes, F=45056]
(group g covers core-flat pixels [g*F, (g+1)*F); tail of group 5 is padding
with logits (+40, -40 x 18) -> p = (1.0, 0...) exactly in bf16, corrected
exactly on host).

Per 4096-pixel tile (per 2048-half):
  ACT  e = exp(l)                          -> bf16
  PE   Z[(g,q), m] = sum_c e[(g,c), q*512+m]   (4 accumulated block matmuls
       with [114, 24] one-hot stationaries -> one PSUM bank [24, 512])
  ACT  m = ln(Z)  [24, 512] fp32
  ACT  m_hi = bf16(m); DVE m_lo = bf16(m - m_hi)
  DMA  reshape m_hi/m_lo [24, 512] -> [6, 2048]
  PE   d = -bc@m_hi - bc@m_lo + I@l    (3 matmuls accumulated in PSUM, fp32)
  ACT  p = exp(d) -> bf16, accum_out = sum p  (conf fold 0, free)
  folds over bf16 p:
    DVE (4x mode)  N_i = sum [p > t_i] and R_i = sum max(p,t_i)-t_i, i=1..7
    ACT            R_8, R_9 via Relu(bias=-t) + accum
    GPSIMD         N_8, N_9 via tensor_scalar(is_gt, add)
m = ln(Z) is DMA'd back to the host, which computes the true-class
p* = exp(l* - m) itself and bins it against target for the accuracy
histogram (tiny), then applies the loss formula.
"""

import numpy as np

import bass_rust
import concourse.bass as bass
from concourse import bacc
import concourse.mybir as mybir
import concourse.tile as tile
from concourse.vector_clock import ScopedClock
from concourse.bass_utils import run_bass_kernel_spmd

F32 = mybir.dt.float32
BF16 = mybir.dt.bfloat16
AF = mybir.ActivationFunctionType
ALU = mybir.AluOpType

# ---------------- problem geometry (hardcoded) ----------------
C = 19
NB = 10
H, W = 1024, 2048
NCORES = 8
ROWS = H // NCORES          # 128
NPIX = ROWS * W             # 262144 valid pixels per core
G = 6
P = G * C                   # 114 partitions
TILE_F = 4096
NT = 11
F = NT * TILE_F             # 45056
NPAD = G * F - NPIX         # 8192 pad pixels
VALID_J5 = NPIX - 5 * F     # 36864 valid pixels in group 5
PAD_TILE0 = VALID_J5 // TILE_F  # = 9; tiles 9,10 have group 5 all-pad

THR = [np.float32(i / 10.0) for i in range(10)]
PADLO, PADHI = -40.0, 40.0

# fold slot layout per tile: [N1..N7, R1..R7] DVE, [R8, R9] ACT, [N8, N9] GP,
# [R0 x 4] ACT exp-accum chunks
NSLOT = 22
SL_N_DVE = 0     # N1..N7 at 0..6
SL_R_DVE = 7     # R1..R7 at 7..13
SL_R_ACT = 14    # R8, R9
SL_N_GP = 16     # N8, N9
SL_R0 = 18       # 4 cols

MM_CHUNK = 512
MCOLS = NT * 1024           # m cols: (t, h, c512) -> t*1024 + h*512 + c

_BUILD_CACHE = {}


def _patch_tile_drain():
    """walrus rejects drains with >1 sync wait; split the tile-exit drain."""
    if getattr(tile.TileContext, "_drain_patched", False):
        return

    def _drain_and_barrier(self, tick_clock, wait_clock):
        drain_inst = self.nc.sync.drain()
        wait_clock.add_sem_waits(
            drain_inst.ins, ScopedClock({None: tick_clock.global_clock})
        )
        si = drain_inst.ins.sync_info
        if si is not None and len(si.on_wait) > 1:
            waits = list(si.on_wait)
            ups = list(si.on_update)
            drain_inst.ins.sync_info = mybir.SyncInfo(on_wait=waits[:1], on_update=[])
            last = drain_inst
            for i in range(1, len(waits)):
                last = self.nc.sync.drain()
                last.ins.sync_info = mybir.SyncInfo(on_wait=waits[i:i + 1], on_update=[])
            if ups:
                lw = list(last.ins.sync_info.on_wait) if last.ins.sync_info else []
                last.ins.sync_info = mybir.SyncInfo(on_wait=lw, on_update=ups)
        self.nc.all_engine_barrier()
        assert self.sems is not None
        popped = self.nc._tile_sem_poison_stack.pop()
        assert popped is self._sem_poison
        self.nc.clear_and_free_semaphores(list(self.sems.allocated().values()))
        self.nc.all_engine_barrier()

    tile.TileContext._drain_and_barrier = _drain_and_barrier
    tile.TileContext._drain_patched = True


def build_nc():
    _patch_tile_drain()
    nc = bacc.Bacc()

    # register threshold constants (ACT bias operands) as const APs
    for v in (float(-THR[8]), float(-THR[9])):
        if (F32, v) not in nc.const_aps.aps:
            tns = nc.alloc_sbuf_tensor(f"const-thr-{v}", [128, 1], F32)
            nc.gpsimd.memset(tns.ap(), v)
            nc.const_aps.aps[(F32, v)] = tns.ap()
    nc.all_engine_barrier()

    lg = nc.declare_dram_parameter("lg", [C, NPIX], F32, isOutput=False)
    zpad = nc.declare_dram_parameter("zpad", [C, TILE_F], F32, isOutput=False)
    bdq = nc.declare_dram_parameter("bdq", [P, 4 * 24], BF16, isOutput=False)
    bcneg = nc.declare_dram_parameter("bcneg", [G, P], BF16, isOutput=False)
    ident = nc.declare_dram_parameter("ident", [P, P], F32, isOutput=False)
    folds_out = nc.declare_dram_parameter("folds", [P, NT * NSLOT], F32, isOutput=True)
    m_out = nc.declare_dram_parameter("mlog", [24, MCOLS], F32, isOutput=True)

    with tile.TileContext(nc) as tc:
        with (
            tc.tile_pool(name="const", bufs=1) as constp,
            tc.tile_pool(name="lt", bufs=2) as lp,
            tc.tile_pool(name="et", bufs=2) as ep,
            tc.tile_pool(name="pt", bufs=2) as pp,
            tc.tile_pool(name="msb", bufs=2) as msp,
            tc.tile_pool(name="mt", bufs=2) as mp,
            tc.tile_pool(name="acc", bufs=1) as accp,
            tc.tile_pool(name="zpsum", bufs=2, space="PSUM") as zp,
            tc.tile_pool(name="dpsum", bufs=2, space="PSUM") as dp,
        ):
            bdq_sb = constp.tile([P, 4 * 24], BF16)
            nc.gpsimd.dma_start(out=bdq_sb[:], in_=bdq[:])
            bc_sb = constp.tile([G, P], BF16)
            nc.gpsimd.dma_start(out=bc_sb[:], in_=bcneg[:])
            id_sb = constp.tile([P, P], F32)
            nc.gpsimd.dma_start(out=id_sb[:], in_=ident[:])

            foldacc = accp.tile([P, NT * NSLOT], F32)
            scr_dve = accp.tile([P, TILE_F], BF16)
            scr_act = accp.tile([P, TILE_F], BF16)
            scr_gp = accp.tile([P, TILE_F], BF16)

            for t in range(NT):
                # ---- load logits tile [114, TILE_F] ----
                lt = lp.tile([P, TILE_F], F32)
                ng = G if t < PAD_TILE0 else G - 1
                base = lg[:, t * TILE_F:(t + 1) * TILE_F]
                src3 = bass_rust.AP(
                    tensor=base.tensor, offset=base.offset,
                    ap=[[F, ng]] + list(base.ap))
                nc.gpsimd.dma_start(out=lt[0:C * ng, :], in_=src3)
                if ng < G:
                    nc.gpsimd.dma_start(out=lt[C * 5:P, :], in_=zpad[:])

                # ---- e = exp(l) -> bf16 ----
                et = ep.tile([P, TILE_F], BF16)
                nc.scalar.activation(et[:], lt[:], AF.Exp)

                pt = pp.tile([P, TILE_F], BF16)
                base_sl = t * NSLOT

                # ---- per 2048-half: Z, ln, m split, d, p ----
                for h in range(2):
                    # Z as [24 = (g, q), 512]: 4 accumulated block matmuls
                    zps = zp.tile([24, MM_CHUNK], F32)
                    for q in range(4):
                        c0 = h * 2048 + q * MM_CHUNK
                        nc.tensor.matmul(
                            zps[:],
                            bdq_sb[:, q * 24:(q + 1) * 24],
                            et[:, c0:c0 + MM_CHUNK],
                            start=(q == 0), stop=(q == 3),
                        )
                    # m = ln(Z) [24, 512] fp32
                    msb = msp.tile([24, MM_CHUNK], F32)
                    nc.scalar.activation(msb[:], zps[:], AF.Ln)
                    nc.gpsimd.dma_start(
                        out=m_out[:, t * 1024 + h * 512:t * 1024 + (h + 1) * 512],
                        in_=msb[:],
                    )
                    # split m = m_hi + m_lo (both bf16)
                    mhi_s = msp.tile([24, MM_CHUNK], BF16)
                    nc.scalar.activation(mhi_s[:], msb[:], AF.Copy)
                    mlo_s = msp.tile([24, MM_CHUNK], BF16)
                    nc.vector.tensor_sub(mlo_s[:], msb[:], mhi_s[:])
                    # m_hi/m_lo to [6, 2048] layout for the broadcast matmul
                    mhi = mp.tile([G, 2048], BF16)
                    nc.gpsimd.dma_start(
                        out=mhi[:].rearrange("g (q c) -> g q c", q=4),
                        in_=mhi_s[:],
                    )
                    mlo = mp.tile([G, 2048], BF16)
                    nc.gpsimd.dma_start(
                        out=mlo[:].rearrange("g (q c) -> g q c", q=4),
                        in_=mlo_s[:],
                    )
                    # d = -bc@m_hi - bc@m_lo + I@l in PSUM, then p = exp(d)
                    for hh in range(2):
                        dps = dp.tile([P, 1024], F32)
                        for q in range(2):
                            mc0 = hh * 1024 + q * MM_CHUNK
                            c0 = h * 2048 + mc0
                            out_sl = dps[:, q * MM_CHUNK:(q + 1) * MM_CHUNK]
                            nc.tensor.matmul(
                                out_sl, bc_sb[:], mhi[:, mc0:mc0 + MM_CHUNK],
                                start=True, stop=False,
                            )
                            nc.tensor.matmul(
                                out_sl, bc_sb[:], mlo[:, mc0:mc0 + MM_CHUNK],
                                start=False, stop=False,
                            )
                            nc.tensor.matmul(
                                out_sl, id_sb[:], lt[:, c0:c0 + MM_CHUNK],
                                start=False, stop=True,
                            )
                        d0 = h * 2048 + hh * 1024
                        nc.scalar.activation(
                            pt[:, d0:d0 + 1024], dps[:], AF.Exp,
                            accum_out=foldacc[:, base_sl + SL_R0 + h * 2 + hh:
                                              base_sl + SL_R0 + h * 2 + hh + 1],
                        )

                # ---- folds over bf16 p ----
                for i in range(1, 8):    # counts N1..N7 on DVE (4x mode)
                    nc.vector.tensor_scalar(
                        scr_dve[:], pt[:], float(THR[i]), None, ALU.is_gt,
                        ALU.add,
                        accum_out=foldacc[:, base_sl + SL_N_DVE + i - 1:
                                          base_sl + SL_N_DVE + i],
                    )
                for i in range(1, 8):    # conf R1..R7 on DVE (4x mode)
                    nc.vector.tensor_scalar(
                        scr_dve[:], pt[:], float(THR[i]), float(THR[i]),
                        ALU.max, ALU.subtract,
                        accum_out=foldacc[:, base_sl + SL_R_DVE + i - 1:
                                          base_sl + SL_R_DVE + i],
                    )
                for k, i in enumerate((8, 9)):   # conf R8, R9 on ACT
                    nc.scalar.activation(
                        scr_act[:], pt[:], AF.Relu, bias=-float(THR[i]),
                        accum_out=foldacc[:, base_sl + SL_R_ACT + k:
                                          base_sl + SL_R_ACT + k + 1],
                    )
                for k, i in enumerate((8, 9)):   # counts N8, N9 on GPSIMD
                    nc.gpsimd.tensor_scalar(
                        scr_gp[:], pt[:], float(THR[i]), None, ALU.is_gt,
                        ALU.add,
                        accum_out=foldacc[:, base_sl + SL_N_GP + k:
                                          base_sl + SL_N_GP + k + 1],
                    )

            # ---- end phase ----
            nc.gpsimd.dma_start(out=folds_out[:], in_=foldacc[:])

    nc.finalize()
    return nc


def _make_consts():
    # bdq: 4 stationaries [114, 24]; block q maps class-group g of chunk q
    # to output row (g, q)
    bdq = np.zeros((P, 4 * 24), np.float32)
    for q in range(4):
        for g in range(G):
            bdq[C * g:C * (g + 1), q * 24 + g * 4 + q] = 1.0
    bc = np.zeros((G, P), np.float32)
    for g in range(G):
        bc[g, C * g:C * (g + 1)] = -1.0
    return bdq, bc


def _shard_host(output: np.ndarray, target: np.ndarray):
    o = np.ascontiguousarray(output[0])          # [19, 1024, 2048]
    bdq, bc = _make_consts()
    zp = np.full((C, TILE_F), PADLO, np.float32)
    zp[0, :] = PADHI

    in_maps = []
    for core in range(NCORES):
        r0 = core * ROWS
        lgc = np.ascontiguousarray(o[:, r0:r0 + ROWS, :].reshape(C, NPIX))
        in_maps.append({
            "lg": lgc, "bdq": bdq, "bcneg": bc,
            "ident": np.eye(P, dtype=np.float32),
            "zpad": zp,
        })
    return in_maps


def _m_to_flat(m: np.ndarray) -> np.ndarray:
    """[24, MCOLS] (g,q)-major -> core-flat [G*F] (incl pad)."""
    return (m.reshape(G, 4, NT, 2, 512).transpose(0, 2, 3, 1, 4).reshape(-1))


def _decode_and_loss(results, output: np.ndarray, target: np.ndarray):
    conf = np.zeros((C, NB), np.float64)
    cnt = np.zeros((C, NB), np.float64)
    acc = np.zeros((C, NB), np.float64)

    o = output[0]
    lstar_full = np.take_along_axis(o, target[0][None].astype(np.int64), axis=0)[0]

    for core in range(NCORES):
        folds = results[core]["folds"].astype(np.float64)
        folds = folds.reshape(P, NT, NSLOT).sum(axis=1)      # [114, 22]
        folds = folds.reshape(G, C, NSLOT).sum(axis=0)       # [C, 22]
        Ni = np.concatenate(
            [np.full((C, 1), float(NPIX)),
             folds[:, SL_N_DVE:SL_N_DVE + 7],
             folds[:, SL_N_GP:SL_N_GP + 2]], axis=1)         # [C, 10]
        R = np.concatenate(
            [folds[:, SL_R0:SL_R0 + 4].sum(axis=1, keepdims=True),
             folds[:, SL_R_DVE:SL_R_DVE + 7],
             folds[:, SL_R_ACT:SL_R_ACT + 2]], axis=1)       # [C, 10]

        # pad pixels land on class 0 with p = 1.0 exactly: remove them
        Ni[0, 1:] -= NPAD
        tgrid = np.arange(10, dtype=np.float64) / 10.0
        R[0, :] -= NPAD * (1.0 - tgrid)

        S = R + tgrid[None, :] * Ni              # S_i = sum p * [p > t_i]
        Snext = np.concatenate([S[:, 1:], np.zeros((C, 1))], axis=1)
        Nnext = np.concatenate([Ni[:, 1:], np.zeros((C, 1))], axis=1)
        conf += S - Snext
        cnt += Ni - Nnext

        # accuracy histogram from host-side p* = exp(l* - m)
        r0 = core * ROWS
        m = _m_to_flat(results[core]["mlog"])[:NPIX]
        ls = lstar_full[r0:r0 + ROWS, :].reshape(-1)
        ps = np.exp(ls - m).astype(np.float32)
        y = target[0, r0:r0 + ROWS, :].reshape(-1)
        b = np.clip(np.ceil(ps * np.float32(10.0)).astype(np.int32) - 1, 0, NB - 1)
        acc += np.bincount(y * NB + b, minlength=C * NB).reshape(C, NB)

    EPS = 1e-13
    avg_acc = acc / (cnt + EPS)
    avg_conf = conf / (cnt + EPS)
    loss = np.sum((avg_acc - avg_conf) ** 2 * (cnt / cnt.sum()))
    return np.float32(loss), (conf, cnt, acc)


def kernel(output: np.ndarray, target: np.ndarray) -> np.ndarray:
    output = np.asarray(output, np.float32)
    target = np.asarray(target, np.int32)
    if "nc" not in _BUILD_CACHE:
        _BUILD_CACHE["nc"] = build_nc()
    nc = _BUILD_CACHE["nc"]
    in_maps = _shard_host(output, target)
    res = run_bass_kernel_spmd(nc, in_maps, list(range(NCORES)))
    loss, _ = _decode_and_loss(res.results, output, target)
    return np.float32(loss)


# revision 8
# speedup vs baseline: 2.0692x; 1.4156x over previous
"""Trainium2 Bass kernel for nn_CCELoss (calibration-histogram loss).

Sharding: data-parallel over image rows, 8 NeuronCores, 128 rows each.

Per-core layout: logits as [114 = 6 pixel-groups x 19 classes, F=45056]
(group g covers core-flat pixels [g*F, (g+1)*F); tail of group 5 is padding
with logits (+40, -40 x 18) -> p = (1.0, 0...) exactly in bf16, corrected
exactly on host).

Three super-blocks of 4/4/3 tiles to minimize ACT table switches (exp and
ln live in different activation-table sets; only copy/relu/sign are in
both):
  Phase A (per tile): ACT e = exp(l) bf16; PE Z via 16 accumulated one-hot
    block matmuls per half -> PSUM [96 = (g, qq128), 256]; ACT copy -> Zbuf
    [96, 2816] (copy needs no table switch).
  Phase B (per super-block): ACT m = ln(Z) in place (one table switch);
    DMA m to host; ACT m_hi = bf16(m); GPSIMD m_lo = bf16(m - m_hi).
  Phase C (per tile): DMA m_hi/m_lo -> [6, .]; PE d = -bc@m_hi - bc@m_lo
    + I@l (fp32r identity, accumulated in PSUM); ACT p = exp(d) bf16 with
    accum_out = sum p (one table switch back); folds.
Folds over bf16 p: DVE 4x-mode tensor_scalar for N1..N7, R1..R8
(is_gt/max-sub with fused accumulate), ACT Relu+accum for R9, GPSIMD for
N8, N9.  All data DMAs issue from the SP sequencer (HWDGE) so the Pool
engine only runs its folds.

The host computes the true-class p* = exp(l* - m) from the returned m,
bins it against target for the accuracy histogram, removes the padding
contribution exactly, and applies the loss formula.
"""

import numpy as np

import bass_rust
import concourse.bass as bass
from concourse import bacc
import concourse.mybir as mybir
import concourse.tile as tile
from concourse.vector_clock import ScopedClock
from concourse.bass_utils import run_bass_kernel_spmd

F32 = mybir.dt.float32
F32R = mybir.dt.float32r
BF16 = mybir.dt.bfloat16
AF = mybir.ActivationFunctionType
ALU = mybir.AluOpType

# ---------------- problem geometry (hardcoded) ----------------
C = 19
NB = 10
H, W = 1024, 2048
NCORES = 8
ROWS = H // NCORES          # 128
NPIX = ROWS * W             # 262144 valid pixels per core
G = 6
P = G * C                   # 114 partitions
TILE_F = 4096
NT = 11
F = NT * TILE_F             # 45056
NPAD = G * F - NPIX         # 8192 pad pixels
VALID_J5 = NPIX - 5 * F     # 36864 valid pixels in group 5
PAD_TILE0 = VALID_J5 // TILE_F  # = 9; tiles 9,10 have group 5 all-pad

THR = [np.float32(i / 10.0) for i in range(10)]
PADLO, PADHI = -40.0, 40.0

# fold slot layout per tile: [N1..N7, R1..R8] DVE, [R9] ACT, [N8, N9] GP,
# [R0 x 4] ACT exp-accum chunks
NSLOT = 22
SL_N_DVE = 0     # N1..N7 at 0..6
SL_R_DVE = 7     # R1..R8 at 7..14
SL_R_ACT = 15    # R9
SL_N_GP = 16     # N8, N9
SL_R0 = 18       # 4 cols

MM_CHUNK = 512
PCOLS = NT * 256            # m cols: (t, h, c128) -> t*256 + h*128 + c
SBS = [(0, 2), (2, 5), (5, 8), (8, NT)]     # super-blocks

_BUILD_CACHE = {}


def _patch_tile_drain():
    """walrus rejects drains with >1 sync wait; split the tile-exit drain."""
    if getattr(tile.TileContext, "_drain_patched", False):
        return

    def _drain_and_barrier(self, tick_clock, wait_clock):
        drain_inst = self.nc.sync.drain()
        wait_clock.add_sem_waits(
            drain_inst.ins, ScopedClock({None: tick_clock.global_clock})
        )
        si = drain_inst.ins.sync_info
        if si is not None and len(si.on_wait) > 1:
            waits = list(si.on_wait)
            ups = list(si.on_update)
            drain_inst.ins.sync_info = mybir.SyncInfo(on_wait=waits[:1], on_update=[])
            last = drain_inst
            for i in range(1, len(waits)):
                last = self.nc.sync.drain()
                last.ins.sync_info = mybir.SyncInfo(on_wait=waits[i:i + 1], on_update=[])
            if ups:
                lw = list(last.ins.sync_info.on_wait) if last.ins.sync_info else []
                last.ins.sync_info = mybir.SyncInfo(on_wait=lw, on_update=ups)
        self.nc.all_engine_barrier()
        assert self.sems is not None
        popped = self.nc._tile_sem_poison_stack.pop()
        assert popped is self._sem_poison
        self.nc.clear_and_free_semaphores(list(self.sems.allocated().values()))
        self.nc.all_engine_barrier()

    tile.TileContext._drain_and_barrier = _drain_and_barrier
    tile.TileContext._drain_patched = True


def build_nc():
    _patch_tile_drain()
    nc = bacc.Bacc()

    # register threshold constants (ACT bias operands) as const APs
    for v in (float(-THR[9]),):
        if (F32, v) not in nc.const_aps.aps:
            tns = nc.alloc_sbuf_tensor(f"const-thr-{v}", [128, 1], F32)
            nc.gpsimd.memset(tns.ap(), v)
            nc.const_aps.aps[(F32, v)] = tns.ap()
    nc.all_engine_barrier()

    lg = nc.declare_dram_parameter("lg", [C, NPIX], F32, isOutput=False)
    zpad = nc.declare_dram_parameter("zpad", [C, TILE_F], F32, isOutput=False)
    bdq = nc.declare_dram_parameter("bdq", [P, 16 * 96], BF16, isOutput=False)
    bcneg = nc.declare_dram_parameter("bcneg", [G, P], BF16, isOutput=False)
    ident = nc.declare_dram_parameter("ident", [P, P], F32, isOutput=False)
    folds_out = nc.declare_dram_parameter("folds", [P, NT * NSLOT], F32, isOutput=True)
    m_out = nc.declare_dram_parameter("mlog", [96, PCOLS], F32, isOutput=True)

    with tile.TileContext(nc) as tc:
        with (
            tc.tile_pool(name="const", bufs=1) as constp,
            tc.tile_pool(name="lt", bufs=5) as lp,
            tc.tile_pool(name="et", bufs=2) as ep,
            tc.tile_pool(name="pt", bufs=3) as pp,
            tc.tile_pool(name="m6", bufs=2) as mp,
            tc.tile_pool(name="acc", bufs=1) as accp,
            tc.tile_pool(name="zpsum", bufs=2, space="PSUM") as zp,
            tc.tile_pool(name="dpsum", bufs=3, space="PSUM") as dp,
        ):
            bdq_sb = constp.tile([P, 16 * 96], BF16)
            nc.sync.dma_start(out=bdq_sb[:], in_=bdq[:])
            bc_sb = constp.tile([G, P], BF16)
            nc.sync.dma_start(out=bc_sb[:], in_=bcneg[:])
            id_sb = constp.tile([P, P], F32)
            nc.sync.dma_start(out=id_sb[:], in_=ident[:])

            foldacc = accp.tile([P, NT * NSLOT], F32)
            zbuf = accp.tile([96, PCOLS], F32)    # Z, then ln(Z) in place
            mhi = accp.tile([96, PCOLS], BF16)
            mlo = accp.tile([96, PCOLS], BF16)
            scr_dve = accp.tile([P, TILE_F], BF16)
            scr_act = accp.tile([P, TILE_F], BF16)
            scr_gp = accp.tile([P, TILE_F], BF16)

            lts = {}
            for sb0, sb1 in SBS:
                # ---- phase A: load, exp, Z ----
                for t in range(sb0, sb1):
                    lt = lp.tile([P, TILE_F], F32)
                    lts[t] = lt
                    ng = G if t < PAD_TILE0 else G - 1
                    base = lg[:, t * TILE_F:(t + 1) * TILE_F]
                    src3 = bass_rust.AP(
                        tensor=base.tensor, offset=base.offset,
                        ap=[[F, ng]] + list(base.ap))
                    nc.sync.dma_start(out=lt[0:C * ng, :], in_=src3)
                    if ng < G:
                        nc.sync.dma_start(out=lt[C * 5:P, :], in_=zpad[:])

                    et = ep.tile([P, TILE_F], BF16)
                    nc.scalar.activation(et[:], lt[:], AF.Exp)

                    # Z[(g, qq), h*128 + c] for the whole tile in one PSUM
                    # tile [96, 256]; 16 accumulated matmuls per half
                    zps = zp.tile([96, 256], F32)
                    for h in range(2):
                        for qq in range(16):
                            c0 = h * 2048 + qq * 128
                            nc.tensor.matmul(
                                zps[:, h * 128:(h + 1) * 128],
                                bdq_sb[:, qq * 96:(qq + 1) * 96],
                                et[:, c0:c0 + 128],
                                start=(qq == 0), stop=(qq == 15),
                            )
                    nc.scalar.activation(
                        zbuf[:, t * 256:(t + 1) * 256], zps[:], AF.Copy,
                    )

                # ---- phase B: ln + split (one table switch to ln set) ----
                sl = slice(sb0 * 256, sb1 * 256)
                nc.scalar.activation(zbuf[:, sl], zbuf[:, sl], AF.Ln)
                nc.sync.dma_start(out=m_out[:, sl], in_=zbuf[:, sl])
                nc.scalar.activation(mhi[:, sl], zbuf[:, sl], AF.Copy)
                nc.gpsimd.tensor_sub(mlo[:, sl], zbuf[:, sl], mhi[:, sl])

                # ---- phase C: d matmuls, exp(d), folds (switch back to exp) --
                for t in range(sb0, sb1):
                    lt = lts.pop(t)
                    # m6: [6, 2*4096] bf16; col = s*4096 + qq*256 + h*128 + c
                    # (row-major unfold of mhi/mlo rows; the matmul moving
                    # operand for a (qt, h) 512-chunk is the strided AP
                    # [4 x 128 @ stride 256] at offset qt*1024 + h*128)
                    m6 = mp.tile([G, 2 * TILE_F], BF16)
                    msl = slice(t * 256, (t + 1) * 256)
                    nc.sync.dma_start(out=m6[:, 0:TILE_F], in_=mhi[:, msl])
                    nc.sync.dma_start(out=m6[:, TILE_F:2 * TILE_F],
                                      in_=mlo[:, msl])
                    m6ap = m6[:]

                    def m6_mov(off):
                        return bass_rust.AP(
                            tensor=m6ap.tensor, offset=m6ap.offset + off,
                            ap=[list(m6ap.ap[0]), [256, 4], [1, 128]])

                    pt = pp.tile([P, TILE_F], BF16)
                    base_sl = t * NSLOT
                    for hb in range(4):      # four [114, 1024] PSUM blocks
                        dps = dp.tile([P, 1024], F32)
                        for q in range(2):
                            n0 = hb * 1024 + q * MM_CHUNK
                            h, qt = n0 // 2048, (n0 % 2048) // MM_CHUNK
                            mcol = qt * 1024 + h * 128
                            out_sl = dps[:, q * MM_CHUNK:(q + 1) * MM_CHUNK]
                            nc.tensor.matmul(
                                out_sl, bc_sb[:], m6_mov(mcol),
                                start=True, stop=False,
                            )
                            nc.tensor.matmul(
                                out_sl, bc_sb[:], m6_mov(TILE_F + mcol),
                                start=False, stop=False,
                            )
                            nc.tensor.matmul(
                                out_sl, id_sb[:].bitcast(F32R),
                                lt[:, n0:n0 + MM_CHUNK].bitcast(F32R),
                                start=False, stop=True,
                            )
                        nc.scalar.activation(
                            pt[:, hb * 1024:(hb + 1) * 1024], dps[:], AF.Exp,
                            accum_out=foldacc[:, base_sl + SL_R0 + hb:
                                              base_sl + SL_R0 + hb + 1],
                        )

                    # ---- folds over bf16 p ----
                    for i in range(1, 8):    # counts N1..N7 on DVE (4x mode)
                        nc.vector.tensor_scalar(
                            scr_dve[:], pt[:], float(THR[i]), None, ALU.is_gt,
                            ALU.add,
                            accum_out=foldacc[:, base_sl + SL_N_DVE + i - 1:
                                              base_sl + SL_N_DVE + i],
                        )
                    for i in range(1, 9):    # conf R1..R8 on DVE (4x mode)
                        nc.vector.tensor_scalar(
                            scr_dve[:], pt[:], float(THR[i]), float(THR[i]),
                            ALU.max, ALU.subtract,
                            accum_out=foldacc[:, base_sl + SL_R_DVE + i - 1:
                                              base_sl + SL_R_DVE + i],
                        )
                    nc.scalar.activation(   # conf R9 on ACT
                        scr_act[:], pt[:], AF.Relu, bias=-float(THR[9]),
                        accum_out=foldacc[:, base_sl + SL_R_ACT:
                                          base_sl + SL_R_ACT + 1],
                    )
                    for k, i in enumerate((8, 9)):   # counts N8, N9 on GPSIMD
                        nc.gpsimd.tensor_scalar(
                            scr_gp[:], pt[:], float(THR[i]), None, ALU.is_gt,
                            ALU.add,
                            accum_out=foldacc[:, base_sl + SL_N_GP + k:
                                              base_sl + SL_N_GP + k + 1],
                        )

            # ---- end phase ----
            nc.sync.dma_start(out=folds_out[:], in_=foldacc[:])

    nc.finalize()
    return nc


def _make_consts():
    # bdq: 16 stationaries [114, 96]; block qq maps class-group g of
    # moving chunk qq*128 to output row (g, qq)
    bdq = np.zeros((P, 16 * 96), np.float32)
    for qq in range(16):
        for g in range(G):
            bdq[C * g:C * (g + 1), qq * 96 + g * 16 + qq] = 1.0
    bc = np.zeros((G, P), np.float32)
    for g in range(G):
        bc[g, C * g:C * (g + 1)] = -1.0
    return bdq, bc


def _shard_host(output: np.ndarray, target: np.ndarray):
    o = np.ascontiguousarray(output[0])          # [19, 1024, 2048]
    bdq, bc = _make_consts()
    zp = np.full((C, TILE_F), PADLO, np.float32)
    zp[0, :] = PADHI

    in_maps = []
    for core in range(NCORES):
        r0 = core * ROWS
        lgc = np.ascontiguousarray(o[:, r0:r0 + ROWS, :].reshape(C, NPIX))
        in_maps.append({
            "lg": lgc, "bdq": bdq, "bcneg": bc,
            "ident": np.eye(P, dtype=np.float32),
            "zpad": zp,
        })
    return in_maps


def _m_to_flat(m: np.ndarray) -> np.ndarray:
    """[96, PCOLS] (g,qq)-major -> core-flat [G*F] (incl pad)."""
    return (m.reshape(G, 16, NT, 2, 128).transpose(0, 2, 3, 1, 4).reshape(-1))


def _decode_and_loss(results, output: np.ndarray, target: np.ndarray):
    conf = np.zeros((C, NB), np.float64)
    cnt = np.zeros((C, NB), np.float64)
    acc = np.zeros((C, NB), np.float64)

    o = output[0]
    lstar_full = np.take_along_axis(o, target[0][None].astype(np.int64), axis=0)[0]

    for core in range(NCORES):
        folds = results[core]["folds"].astype(np.float64)
        folds = folds.reshape(P, NT, NSLOT).sum(axis=1)      # [114, 22]
        folds = folds.reshape(G, C, NSLOT).sum(axis=0)       # [C, 22]
        Ni = np.concatenate(
            [np.full((C, 1), float(NPIX)),
             folds[:, SL_N_DVE:SL_N_DVE + 7],
             folds[:, SL_N_GP:SL_N_GP + 2]], axis=1)         # [C, 10]
        R = np.concatenate(
            [folds[:, SL_R0:SL_R0 + 4].sum(axis=1, keepdims=True),
             folds[:, SL_R_DVE:SL_R_DVE + 8],
             folds[:, SL_R_ACT:SL_R_ACT + 1]], axis=1)       # [C, 10]

        # pad pixels land on class 0 with p = 1.0 exactly: remove them
        Ni[0, 1:] -= NPAD
        tgrid = np.arange(10, dtype=np.float64) / 10.0
        R[0, :] -= NPAD * (1.0 - tgrid)

        S = R + tgrid[None, :] * Ni              # S_i = sum p * [p > t_i]
        Snext = np.concatenate([S[:, 1:], np.zeros((C, 1))], axis=1)
        Nnext = np.concatenate([Ni[:, 1:], np.zeros((C, 1))], axis=1)
        conf += S - Snext
        cnt += Ni - Nnext

        # accuracy histogram from host-side p* = exp(l* - m)
        r0 = core * ROWS
        m = _m_to_flat(results[core]["mlog"])[:NPIX]
        ls = lstar_full[r0:r0 + ROWS, :].reshape(-1)
        ps = np.exp(ls - m).astype(np.float32)
        y = target[0, r0:r0 + ROWS, :].reshape(-1)
        b = np.clip(np.ceil(ps * np.float32(10.0)).astype(np.int32) - 1, 0, NB - 1)
        acc += np.bincount(y * NB + b, minlength=C * NB).reshape(C, NB)

    EPS = 1e-13
    avg_acc = acc / (cnt + EPS)
    avg_conf = conf / (cnt + EPS)
    loss = np.sum((avg_acc - avg_conf) ** 2 * (cnt / cnt.sum()))
    return np.float32(loss), (conf, cnt, acc)


def kernel(output: np.ndarray, target: np.ndarray) -> np.ndarray:
    output = np.asarray(output, np.float32)
    target = np.asarray(target, np.int32)
    if "nc" not in _BUILD_CACHE:
        _BUILD_CACHE["nc"] = build_nc()
    nc = _BUILD_CACHE["nc"]
    in_maps = _shard_host(output, target)
    res = run_bass_kernel_spmd(nc, in_maps, list(range(NCORES)))
    loss, _ = _decode_and_loss(res.results, output, target)
    return np.float32(loss)
